# revision 8
# baseline (speedup 1.0000x reference)
"""Trainium2 Bass kernel for the BestRQ vq_codebook problem.

Per token: LayerNorm(D=512) -> targets = xn @ proj_w.T (Q=1024)
          -> labels = argmin_c ||targets - codebook[:, c]||  (C=4096)
          -> encoder_out = xn @ enc_w.T + enc_b.

Device strategy (8 cores, data-parallel over the 65536 tokens):
  - All matmuls in float32r (full PE rate, ~13-bit effective mantissa).
  - argmin via score = targets . cb_c - 0.5*||c||^2 (argmax), computed with
    tokens on PSUM partitions and C on the free dim, so the DVE max/max_index
    instructions produce per-token top-8 values + indices.
  - The codebook is processed in two C-halves (SBUF capacity); the host merges
    the halves and exactly rescores (fp64) every token whose coarse top-2
    margin is below a threshold, which absorbs all float32r rounding.
  - targets^T is computed directly (q on partitions) so it is already in lhsT
    layout for the distance matmul; it is spilled to DRAM between phases.
"""

import os
from contextlib import ExitStack

import numpy as np

import concourse.bass as bass
import concourse.masks as masks
import concourse.mybir as mybir
import concourse.tile as tile
from concourse import bacc, bass_utils

F32 = mybir.dt.float32
F32R = mybir.dt.float32r
U32 = mybir.dt.uint32

D = 512          # feature dim
Q = 1024         # projection dim
C = 4096         # codebook size
CHALF = C // 2
TOK_PER_CHUNK = 512
LN_EPS = 1e-5
MARGIN = 0.25    # coarse-score margin below which the host rescores exactly


def build_module(n_chunks, has_eb=True):
    """Emit the per-core module. Every core runs the same program on its own
    token shard (tokens = n_chunks * 512)."""
    tokens = n_chunks * TOK_PER_CHUNK
    nc = bacc.Bacc("TRN2", target_bir_lowering=False, debug=False, num_devices=8)

    x_d = nc.dram_tensor("x", [tokens, D], F32, kind="ExternalInput").ap()
    weff_d = nc.dram_tensor("weff", [D, Q], F32R, kind="ExternalInput").ap()
    eeff_d = nc.dram_tensor("eeff", [D, Q], F32R, kind="ExternalInput").ap()
    wbias_d = nc.dram_tensor("wbias", [128, Q // 128], F32, kind="ExternalInput").ap()
    ebrow_d = nc.dram_tensor("ebrow", [1, Q], F32R, kind="ExternalInput").ap()
    cb_d = nc.dram_tensor("cb", [Q, C], F32R, kind="ExternalInput").ap()
    ncsqb_d = nc.dram_tensor("ncsqb", [128, C], F32, kind="ExternalInput").ap()
    ones_d = nc.dram_tensor("ones", [1, 128], F32R, kind="ExternalInput").ap()

    enc_d = nc.dram_tensor("enc", [tokens, Q], F32, kind="ExternalOutput").ap()
    aux_d = nc.dram_tensor("aux", [2, tokens, 16], F32, kind="ExternalOutput").ap()

    AF = mybir.ActivationFunctionType
    DC = D // 128    # 4 contraction chunks for D
    QC = Q // 128    # 8 q-tiles / contraction chunks for Q

    with tile.TileContext(nc) as tc, ExitStack() as ctx:
        dram = ctx.enter_context(tc.tile_pool(name="dram", bufs=1, space="DRAM"))
        tt_d = dram.tile([Q, tokens], F32R)

        cpool = ctx.enter_context(tc.tile_pool(name="const", bufs=1))
        ident = cpool.tile([128, 128], F32)
        masks.make_identity(nc, ident[:])
        ones = cpool.tile([1, 128], F32R)
        nc.sync.dma_start(ones[:], ones_d[:, :])
        wbias_sb = cpool.tile([128, Q // 128], F32)
        nc.sync.dma_start(wbias_sb[:], wbias_d[:, :])
        ebrow_sb = cpool.tile([1, Q], F32R)
        nc.sync.dma_start(ebrow_sb[:], ebrow_d[:, :])
        ncsqb_sb = cpool.tile([128, C], F32)
        nc.sync.dma_start(ncsqb_sb[:], ncsqb_d[:, :])

        # ---------------- Phase A: LN + transpose + proj + enc ----------------
        with (
            tc.tile_pool(name="wres", bufs=1) as wres,
            tc.tile_pool(name="xa", bufs=6) as xa,
            tc.tile_pool(name="xn", bufs=6) as xnp,
            tc.tile_pool(name="xnT", bufs=5) as xntp,
            tc.tile_pool(name="tts", bufs=10) as tts,
            tc.tile_pool(name="encs", bufs=3) as encs,
            tc.tile_pool(name="scr", bufs=2) as scrp,
            tc.tile_pool(name="stats", bufs=2) as stp,
            tc.tile_pool(name="pst", bufs=2, space="PSUM") as pst,
            tc.tile_pool(name="psj", bufs=2, space="PSUM") as psj,
            tc.tile_pool(name="pse", bufs=2, space="PSUM") as pse,
        ):
            weff_sb = []
            eeff_sb = []
            for dc in range(DC):
                w = wres.tile([128, Q], F32R, tag=f"w{dc}")
                nc.sync.dma_start(w[:], weff_d[dc * 128:(dc + 1) * 128, :])
                weff_sb.append(w)
                e = wres.tile([128, Q], F32R, tag=f"e{dc}")
                nc.sync.dma_start(e[:], eeff_d[dc * 128:(dc + 1) * 128, :])
                eeff_sb.append(e)

            for ch in range(n_chunks):
                t0 = ch * TOK_PER_CHUNK
                xts = []
                sumx = stp.tile([128, 4], F32, tag="sumx")
                sumx2 = stp.tile([128, 4], F32, tag="sumx2")
                for s in range(4):
                    xt = xa.tile([128, D], F32, tag="x")
                    nc.sync.dma_start(xt[:], x_d[t0 + s * 128: t0 + (s + 1) * 128, :])
                    xts.append(xt)
                    scr = scrp.tile([128, D], F32, tag="scr")
                    nc.scalar.activation(scr[:], xt[:], AF.Square,
                                         accum_out=sumx2[:, s:s + 1])
                    scr2 = scrp.tile([128, D], F32, tag="scr")
                    nc.scalar.activation(scr2[:], xt[:], AF.Copy,
                                         accum_out=sumx[:, s:s + 1])

                # batched LN stats for the 4 subtiles: r = rsqrt(var+eps) with
                # one Newton step on top of the ACT sqrt LUT; nrmu = -mu * r.
                mun = stp.tile([128, 4], F32, tag="mun")
                nc.vector.tensor_scalar_mul(mun[:], sumx[:], -1.0 / D)
                m2 = stp.tile([128, 4], F32, tag="m2")
                nc.vector.tensor_mul(m2[:], mun[:], mun[:])
                veps = stp.tile([128, 4], F32, tag="veps")
                nc.vector.tensor_scalar(veps[:], sumx2[:], 1.0 / D, LN_EPS,
                                        op0=mybir.AluOpType.mult,
                                        op1=mybir.AluOpType.add)
                nc.vector.tensor_sub(veps[:], veps[:], m2[:])
                inv = stp.tile([128, 4], F32, tag="inv")
                nc.vector.reciprocal(inv[:], veps[:])
                r0 = stp.tile([128, 4], F32, tag="r0")
                nc.scalar.sqrt(r0[:], inv[:])
                # Newton: r = r0 * (1.5 - 0.5 * veps * r0^2)
                a = stp.tile([128, 4], F32, tag="a")
                nc.vector.tensor_mul(a[:], r0[:], r0[:])
                nc.vector.tensor_mul(a[:], a[:], veps[:])
                nc.vector.tensor_mul(a[:], a[:], r0[:])      # veps * r0^3
                rt = stp.tile([128, 4], F32, tag="rt")
                nc.vector.tensor_scalar_mul(rt[:], r0[:], 1.5)
                nc.vector.tensor_scalar_mul(a[:], a[:], -0.5)
                nc.vector.tensor_add(rt[:], rt[:], a[:])
                nrmu = stp.tile([128, 4], F32, tag="nrmu")
                nc.vector.tensor_mul(nrmu[:], mun[:], rt[:])

                # x_n = r*x - r*mu, then PE-transpose into x_nT chunks
                xnT = []
                for dc in range(DC):
                    xnT.append(xntp.tile([128, TOK_PER_CHUNK], F32R, tag=f"xnT{dc}", name=f"xnT{dc}"))
                for s in range(4):
                    xn = xnp.tile([128, D], F32, tag="xn")
                    nc.scalar.activation(xn[:], xts[s][:], AF.Identity,
                                         bias=nrmu[:, s:s + 1], scale=rt[:, s:s + 1])
                    for dc in range(DC):
                        ptr = pst.tile([128, 128], F32, tag="ptr")
                        nc.tensor.transpose(ptr[:], xn[:, dc * 128:(dc + 1) * 128],
                                            ident[:])
                        nc.scalar.copy(xnT[dc][:, s * 128:(s + 1) * 128], ptr[:])

                # proj: targets^T (q on partitions) + spill to DRAM
                for qt in range(QC):
                    ps = psj.tile([128, TOK_PER_CHUNK], F32, tag="psj")
                    for dc in range(DC):
                        nc.tensor.matmul(ps[:], (weff_sb[dc][:, qt * 128:(qt + 1) * 128]),
                                         (xnT[dc][:]),
                                         start=(dc == 0), stop=(dc == DC - 1))
                    ttt = tts.tile([128, TOK_PER_CHUNK], F32R, tag="tt")
                    nc.scalar.activation(ttt[:], ps[:], AF.Identity,
                                         bias=wbias_sb[:, qt:qt + 1], scale=1.0)
                    nc.sync.dma_start(tt_d[qt * 128:(qt + 1) * 128, t0:t0 + TOK_PER_CHUNK],
                                      ttt[:])

                # enc: natural layout [tok, q], bias via K=1 ones row
                for s in range(4):
                    ps2 = pse.tile([128, Q], F32, tag="pse")
                    for dc in range(DC):
                        for h in range(Q // 512):
                            hs = slice(h * 512, (h + 1) * 512)
                            nc.tensor.matmul(ps2[:, hs],
                                             (xnT[dc][:, s * 128:(s + 1) * 128]),
                                             (eeff_sb[dc][:, hs]),
                                             start=(dc == 0),
                                             stop=(not has_eb and dc == DC - 1))
                    if has_eb:
                        for h in range(Q // 512):
                            hs = slice(h * 512, (h + 1) * 512)
                            nc.tensor.matmul(ps2[:, hs], (ones[:]), (ebrow_sb[:, hs]),
                                             start=False, stop=True)
                    en = encs.tile([128, Q], F32, tag="en")
                    nc.scalar.copy(en[:], ps2[:])
                    nc.sync.dma_start(enc_d[t0 + s * 128: t0 + (s + 1) * 128, :], en[:])

        # ---------------- Phase B/C: distances + argmax per C-half ----------------
        with (
            tc.tile_pool(name="cbp", bufs=1) as cbp,
            tc.tile_pool(name="ttl", bufs=2) as ttlp,
            tc.tile_pool(name="sc", bufs=3) as scp,
            tc.tile_pool(name="st", bufs=4) as sttp,
            tc.tile_pool(name="psb", bufs=2, space="PSUM") as psb,
        ):
            for half in range(2):
                cb_sb = []
                for qc in range(QC):
                    cbt = cbp.tile([128, CHALF], F32R, tag=f"cb{qc}")
                    nc.sync.dma_start(cbt[:], cb_d[qc * 128:(qc + 1) * 128,
                                                   half * CHALF:(half + 1) * CHALF])
                    cb_sb.append(cbt)
                for ch in range(n_chunks):
                    t0 = ch * TOK_PER_CHUNK
                    ttl = []
                    for qc in range(QC):
                        t = ttlp.tile([128, TOK_PER_CHUNK], F32R, tag=f"ttl{qc}")
                        nc.sync.dma_start(t[:], tt_d[qc * 128:(qc + 1) * 128,
                                                     t0:t0 + TOK_PER_CHUNK])
                        ttl.append(t)
                    for s in range(4):
                        ps = psb.tile([128, CHALF], F32, tag="psb")
                        for qc in range(QC):
                            for n in range(CHALF // 512):
                                ns = slice(n * 512, (n + 1) * 512)
                                nc.tensor.matmul(ps[:, ns],
                                                 (ttl[qc][:, s * 128:(s + 1) * 128]),
                                                 (cb_sb[qc][:, ns]),
                                                 start=(qc == 0), stop=(qc == QC - 1))
                        sc = scp.tile([128, CHALF], F32, tag="sc")
                        nc.vector.tensor_add(sc[:], ps[:],
                                             ncsqb_sb[:, half * CHALF:(half + 1) * CHALF])
                        st = sttp.tile([128, 16], F32, tag="st")
                        nc.vector.max(st[:, 0:8], sc[:])
                        nc.vector.max_index(st[:, 8:16].bitcast(U32), st[:, 0:8], sc[:])
                        nc.sync.dma_start(aux_d[half, t0 + s * 128: t0 + (s + 1) * 128, :],
                                          st[:])

    nc.compile()
    return nc


_MODULE_CACHE = {}


def _get_module(n_chunks, has_eb=True):
    key = (n_chunks, has_eb)
    if key not in _MODULE_CACHE:
        _MODULE_CACHE[key] = build_module(n_chunks, has_eb=has_eb)
    return _MODULE_CACHE[key]


def _host_reference_scores(xs, gamma, beta, W, CB, c_sq64):
    """Exact fp64 scores (argmax == argmin of distance) for a token subset."""
    xs = xs.astype(np.float64)
    mu = xs.mean(-1, keepdims=True)
    var = xs.var(-1, keepdims=True)
    xn = (xs - mu) / np.sqrt(var + LN_EPS) * gamma.astype(np.float64) + beta.astype(np.float64)
    t = xn @ W.astype(np.float64).T
    return t @ CB.astype(np.float64) - 0.5 * c_sq64


def kernel(**inputs):
    x = np.asarray(inputs["input_values"], dtype=np.float32)
    gamma = np.asarray(inputs["ln_gamma"], dtype=np.float32)
    beta = np.asarray(inputs["ln_beta"], dtype=np.float32)
    W = np.asarray(inputs["proj_w"], dtype=np.float32)
    CB = np.asarray(inputs["code_book"], dtype=np.float32)
    E = np.asarray(inputs["enc_w"], dtype=np.float32)
    eb = np.asarray(inputs["enc_b"], dtype=np.float32)

    B, T, Dx = x.shape
    assert Dx == D and W.shape == (Q, D) and CB.shape == (Q, C)
    xf = np.ascontiguousarray(x.reshape(-1, D))
    N = xf.shape[0]
    NCORE = 8
    tpc = N // NCORE
    n_chunks = tpc // TOK_PER_CHUNK

    weff = np.ascontiguousarray((W * gamma[None, :]).T)
    eeff = np.ascontiguousarray((E * gamma[None, :]).T)
    wbias = np.ascontiguousarray((W.astype(np.float64) @ beta.astype(np.float64))
                                 .astype(np.float32).reshape(Q // 128, 128).T)
    ebrow = np.ascontiguousarray(
        ((E.astype(np.float64) @ beta.astype(np.float64)) + eb.astype(np.float64))
        .astype(np.float32).reshape(1, Q))
    c_sq64 = (CB.astype(np.float64) ** 2).sum(0)
    ncsqb = np.ascontiguousarray(np.broadcast_to(
        (-0.5 * (c_sq64 - c_sq64.mean())).astype(np.float32)[None, :], (128, C)))

    has_eb = bool(np.any(ebrow))
    nc = _get_module(n_chunks, has_eb=has_eb)
    in_maps = []
    for cidx in range(NCORE):
        in_maps.append({
            "x": np.ascontiguousarray(xf[cidx * tpc:(cidx + 1) * tpc]),
            "weff": weff, "eeff": eeff, "wbias": wbias, "ebrow": ebrow,
            "cb": CB, "ncsqb": ncsqb,
            "ones": np.ones((1, 128), np.float32),
        })

    trace = bool(int(os.environ.get("KERNEL_TRACE", "0")))
    res = bass_utils.run_bass_kernel_spmd(nc, in_maps, core_ids=list(range(NCORE)),
                                          trace=trace)
    kernel.last_results = res

    enc = np.concatenate([np.asarray(r["enc"]) for r in res.results], axis=0)
    aux = np.stack([np.asarray(r["aux"]) for r in res.results], axis=0)  # (8,2,tpc,16)

    v1 = aux[..., 0].astype(np.float64)          # (8, 2, tpc) top-1 per half
    v2 = aux[..., 1].astype(np.float64)          # second-best per half
    idx = np.ascontiguousarray(aux).view(np.uint32)[..., 8].astype(np.int64)

    win = np.argmax(v1, axis=1)                  # (8, tpc) winning half
    ar = np.arange(tpc)
    cr = np.arange(NCORE)[:, None]
    vwin = v1[cr, win, ar]
    vlose = v1[cr, 1 - win, ar]
    v2win = v2[cr, win, ar]
    second = np.maximum(vlose, v2win)
    margin = vwin - second
    labels = (idx[cr, win, ar] + win.astype(np.int64) * CHALF).reshape(-1)

    flagged = (margin < MARGIN).reshape(-1)
    kernel.last_flagged_count = int(flagged.sum())
    if flagged.any():
        fidx = np.nonzero(flagged)[0]
        scores = _host_reference_scores(xf[fidx], gamma, beta, W, CB, c_sq64)
        labels[fidx] = scores.argmax(-1)

    encoder_out = enc.reshape(B, T, Q)
    labels = labels.reshape(B, T).astype(np.int32)
    return encoder_out, labels


# revision 9
# speedup vs baseline: 1.0009x; 1.0009x over previous
"""Trainium2 Bass kernel for the BestRQ vq_codebook problem.

Per token: LayerNorm(D=512) -> targets = xn @ proj_w.T (Q=1024)
          -> labels = argmin_c ||targets - codebook[:, c]||  (C=4096)
          -> encoder_out = xn @ enc_w.T + enc_b.

Device strategy (8 cores, data-parallel over the 65536 tokens):
  - All matmuls in float32r (full PE rate, ~13-bit effective mantissa).
  - argmin via score = targets . cb_c - 0.5*||c||^2 (argmax), computed with
    tokens on PSUM partitions and C on the free dim, so the DVE max/max_index
    instructions produce per-token top-8 values + indices.
  - The codebook is processed in two C-halves (SBUF capacity); the host merges
    the halves and exactly rescores (fp64) every token whose coarse top-2
    margin is below a threshold, which absorbs all float32r rounding.
  - targets^T is computed directly (q on partitions) so it is already in lhsT
    layout for the distance matmul; it is spilled to DRAM between phases.
"""

import os
from contextlib import ExitStack

import numpy as np

import concourse.bass as bass
import concourse.masks as masks
import concourse.mybir as mybir
import concourse.tile as tile
from concourse import bacc, bass_utils

F32 = mybir.dt.float32
F32R = mybir.dt.float32r
U32 = mybir.dt.uint32

D = 512          # feature dim
Q = 1024         # projection dim
C = 4096         # codebook size
CHALF = C // 2
TOK_PER_CHUNK = 512
LN_EPS = 1e-5
MARGIN = 0.25    # coarse-score margin below which the host rescores exactly


def build_module(n_chunks, has_eb=True):
    """Emit the per-core module. Every core runs the same program on its own
    token shard (tokens = n_chunks * 512)."""
    tokens = n_chunks * TOK_PER_CHUNK
    nc = bacc.Bacc("TRN2", target_bir_lowering=False, debug=False, num_devices=8)

    x_d = nc.dram_tensor("x", [tokens, D], F32, kind="ExternalInput").ap()
    weff_d = nc.dram_tensor("weff", [D, Q], F32R, kind="ExternalInput").ap()
    eeff_d = nc.dram_tensor("eeff", [D, Q], F32R, kind="ExternalInput").ap()
    wbias_d = nc.dram_tensor("wbias", [128, Q // 128], F32, kind="ExternalInput").ap()
    ebrow_d = nc.dram_tensor("ebrow", [1, Q], F32R, kind="ExternalInput").ap()
    cb_d = nc.dram_tensor("cb", [Q, C], F32R, kind="ExternalInput").ap()
    ncsqb_d = nc.dram_tensor("ncsqb", [128, C], F32, kind="ExternalInput").ap()
    ones_d = nc.dram_tensor("ones", [1, 128], F32R, kind="ExternalInput").ap()

    enc_d = nc.dram_tensor("enc", [tokens, Q], F32, kind="ExternalOutput").ap()
    aux_d = nc.dram_tensor("aux", [2, tokens, 16], F32, kind="ExternalOutput").ap()

    AF = mybir.ActivationFunctionType
    DC = D // 128    # 4 contraction chunks for D
    QC = Q // 128    # 8 q-tiles / contraction chunks for Q

    with tile.TileContext(nc) as tc, ExitStack() as ctx:
        dram = ctx.enter_context(tc.tile_pool(name="dram", bufs=1, space="DRAM"))
        tt_d = dram.tile([Q, tokens], F32R)

        cpool = ctx.enter_context(tc.tile_pool(name="const", bufs=1))
        ident = cpool.tile([128, 128], F32)
        masks.make_identity(nc, ident[:])
        ones = cpool.tile([1, 128], F32R)
        nc.sync.dma_start(ones[:], ones_d[:, :])
        wbias_sb = cpool.tile([128, Q // 128], F32)
        nc.sync.dma_start(wbias_sb[:], wbias_d[:, :])
        ebrow_sb = cpool.tile([1, Q], F32R)
        nc.sync.dma_start(ebrow_sb[:], ebrow_d[:, :])
        ncsqb_sb = cpool.tile([128, C], F32)
        nc.sync.dma_start(ncsqb_sb[:], ncsqb_d[:, :])

        # ---------------- Phase A: LN + transpose + proj + enc ----------------
        with (
            tc.tile_pool(name="wres", bufs=1) as wres,
            tc.tile_pool(name="xa", bufs=6) as xa,
            tc.tile_pool(name="xn", bufs=6) as xnp,
            tc.tile_pool(name="xnT", bufs=5) as xntp,
            tc.tile_pool(name="tts", bufs=10) as tts,
            tc.tile_pool(name="encs", bufs=3) as encs,
            tc.tile_pool(name="scr", bufs=2) as scrp,
            tc.tile_pool(name="stats", bufs=2) as stp,
            tc.tile_pool(name="pst", bufs=2, space="PSUM") as pst,
            tc.tile_pool(name="psj", bufs=2, space="PSUM") as psj,
            tc.tile_pool(name="pse", bufs=2, space="PSUM") as pse,
        ):
            weff_sb = []
            eeff_sb = []
            for dc in range(DC):
                w = wres.tile([128, Q], F32R, tag=f"w{dc}")
                nc.sync.dma_start(w[:], weff_d[dc * 128:(dc + 1) * 128, :])
                weff_sb.append(w)
                e = wres.tile([128, Q], F32R, tag=f"e{dc}")
                nc.sync.dma_start(e[:], eeff_d[dc * 128:(dc + 1) * 128, :])
                eeff_sb.append(e)

            for ch in range(n_chunks):
                t0 = ch * TOK_PER_CHUNK
                xts = []
                sumx = stp.tile([128, 4], F32, tag="sumx")
                sumx2 = stp.tile([128, 4], F32, tag="sumx2")
                for s in range(4):
                    xt = xa.tile([128, D], F32, tag="x")
                    nc.sync.dma_start(xt[:], x_d[t0 + s * 128: t0 + (s + 1) * 128, :])
                    xts.append(xt)
                    scr = scrp.tile([128, D], F32, tag="scr")
                    nc.scalar.activation(scr[:], xt[:], AF.Square,
                                         accum_out=sumx2[:, s:s + 1])
                    scr2 = scrp.tile([128, D], F32, tag="scr")
                    nc.scalar.activation(scr2[:], xt[:], AF.Copy,
                                         accum_out=sumx[:, s:s + 1])

                # batched LN stats for the 4 subtiles: r = rsqrt(var+eps) with
                # one Newton step on top of the ACT sqrt LUT; nrmu = -mu * r.
                mun = stp.tile([128, 4], F32, tag="mun")
                nc.vector.tensor_scalar_mul(mun[:], sumx[:], -1.0 / D)
                m2 = stp.tile([128, 4], F32, tag="m2")
                nc.vector.tensor_mul(m2[:], mun[:], mun[:])
                veps = stp.tile([128, 4], F32, tag="veps")
                nc.vector.tensor_scalar(veps[:], sumx2[:], 1.0 / D, LN_EPS,
                                        op0=mybir.AluOpType.mult,
                                        op1=mybir.AluOpType.add)
                nc.vector.tensor_sub(veps[:], veps[:], m2[:])
                inv = stp.tile([128, 4], F32, tag="inv")
                nc.vector.reciprocal(inv[:], veps[:])
                r0 = stp.tile([128, 4], F32, tag="r0")
                nc.scalar.sqrt(r0[:], inv[:])
                # Newton: r = r0 * (1.5 - 0.5 * veps * r0^2)
                a = stp.tile([128, 4], F32, tag="a")
                nc.vector.tensor_mul(a[:], r0[:], r0[:])
                nc.vector.tensor_mul(a[:], a[:], veps[:])
                nc.vector.tensor_mul(a[:], a[:], r0[:])      # veps * r0^3
                rt = stp.tile([128, 4], F32, tag="rt")
                nc.vector.tensor_scalar_mul(rt[:], r0[:], 1.5)
                nc.vector.tensor_scalar_mul(a[:], a[:], -0.5)
                nc.vector.tensor_add(rt[:], rt[:], a[:])
                nrmu = stp.tile([128, 4], F32, tag="nrmu")
                nc.vector.tensor_mul(nrmu[:], mun[:], rt[:])

                # x_n = r*x - r*mu, then PE-transpose into x_nT chunks
                xnT = []
                for dc in range(DC):
                    xnT.append(xntp.tile([128, TOK_PER_CHUNK], F32R, tag=f"xnT{dc}", name=f"xnT{dc}"))
                for s in range(4):
                    xn = xnp.tile([128, D], F32, tag="xn")
                    nc.scalar.activation(xn[:], xts[s][:], AF.Identity,
                                         bias=nrmu[:, s:s + 1], scale=rt[:, s:s + 1])
                    for dc in range(DC):
                        ptr = pst.tile([128, 128], F32, tag="ptr")
                        nc.tensor.transpose(ptr[:], xn[:, dc * 128:(dc + 1) * 128],
                                            ident[:])
                        nc.scalar.copy(xnT[dc][:, s * 128:(s + 1) * 128], ptr[:])

                # proj: targets^T (q on partitions) + spill to DRAM
                for qt in range(QC):
                    ps = psj.tile([128, TOK_PER_CHUNK], F32, tag="psj")
                    for dc in range(DC):
                        nc.tensor.matmul(ps[:], (weff_sb[dc][:, qt * 128:(qt + 1) * 128]),
                                         (xnT[dc][:]),
                                         start=(dc == 0), stop=(dc == DC - 1))
                    ttt = tts.tile([128, TOK_PER_CHUNK], F32R, tag="tt")
                    nc.scalar.activation(ttt[:], ps[:], AF.Identity,
                                         bias=wbias_sb[:, qt:qt + 1], scale=1.0)
                    nc.sync.dma_start(tt_d[qt * 128:(qt + 1) * 128, t0:t0 + TOK_PER_CHUNK],
                                      ttt[:])

                # enc: natural layout [tok, q], bias via K=1 ones row
                for s in range(4):
                    ps2 = pse.tile([128, Q], F32, tag="pse")
                    for dc in range(DC):
                        for h in range(Q // 512):
                            hs = slice(h * 512, (h + 1) * 512)
                            nc.tensor.matmul(ps2[:, hs],
                                             (xnT[dc][:, s * 128:(s + 1) * 128]),
                                             (eeff_sb[dc][:, hs]),
                                             start=(dc == 0),
                                             stop=(not has_eb and dc == DC - 1))
                    if has_eb:
                        for h in range(Q // 512):
                            hs = slice(h * 512, (h + 1) * 512)
                            nc.tensor.matmul(ps2[:, hs], (ones[:]), (ebrow_sb[:, hs]),
                                             start=False, stop=True)
                    en = encs.tile([128, Q], F32, tag="en")
                    nc.scalar.copy(en[:], ps2[:])
                    nc.sync.dma_start(enc_d[t0 + s * 128: t0 + (s + 1) * 128, :], en[:])

        # ---------------- Phase B/C: distances + argmax per C-half ----------------
        with (
            tc.tile_pool(name="cbp", bufs=1) as cbp,
            tc.tile_pool(name="ttl", bufs=2) as ttlp,
            tc.tile_pool(name="sc", bufs=3) as scp,
            tc.tile_pool(name="st", bufs=4) as sttp,
            tc.tile_pool(name="psb", bufs=2, space="PSUM") as psb,
        ):
            for half in range(2):
                cb_sb = []
                for qc in range(QC):
                    cbt = cbp.tile([128, CHALF], F32R, tag=f"cb{qc}")
                    nc.sync.dma_start(cbt[:], cb_d[qc * 128:(qc + 1) * 128,
                                                   half * CHALF:(half + 1) * CHALF])
                    cb_sb.append(cbt)
                for ch in range(n_chunks):
                    t0 = ch * TOK_PER_CHUNK
                    ttl = []
                    for qc in range(QC):
                        t = ttlp.tile([128, TOK_PER_CHUNK], F32R, tag=f"ttl{qc}")
                        nc.sync.dma_start(t[:], tt_d[qc * 128:(qc + 1) * 128,
                                                     t0:t0 + TOK_PER_CHUNK])
                        ttl.append(t)
                    for s in range(4):
                        ps = psb.tile([128, CHALF], F32, tag="psb")
                        for qc in range(QC):
                            for n in range(CHALF // 512):
                                ns = slice(n * 512, (n + 1) * 512)
                                nc.tensor.matmul(ps[:, ns],
                                                 (ttl[qc][:, s * 128:(s + 1) * 128]),
                                                 (cb_sb[qc][:, ns]),
                                                 start=(qc == 0), stop=(qc == QC - 1))
                        sc = scp.tile([128, CHALF], F32, tag="sc")
                        nc.vector.tensor_add(sc[:], ps[:],
                                             ncsqb_sb[:, half * CHALF:(half + 1) * CHALF])
                        st = sttp.tile([128, 16], F32, tag="st")
                        nc.vector.max(st[:, 0:8], sc[:])
                        nc.vector.max_index(st[:, 8:16].bitcast(U32), st[:, 0:8], sc[:])
                        nc.sync.dma_start(aux_d[half, t0 + s * 128: t0 + (s + 1) * 128, :],
                                          st[:])

    nc.compile()
    return nc


_MODULE_CACHE = {}


def _get_module(n_chunks, has_eb=True):
    key = (n_chunks, has_eb)
    if key not in _MODULE_CACHE:
        _MODULE_CACHE[key] = build_module(n_chunks, has_eb=has_eb)
    return _MODULE_CACHE[key]


def _host_reference_scores(xs, gamma, beta, W, CB, c_sq64):
    """Exact fp64 scores (argmax == argmin of distance) for a token subset."""
    xs = xs.astype(np.float64)
    mu = xs.mean(-1, keepdims=True)
    var = xs.var(-1, keepdims=True)
    xn = (xs - mu) / np.sqrt(var + LN_EPS) * gamma.astype(np.float64) + beta.astype(np.float64)
    t = xn @ W.astype(np.float64).T
    return t @ CB.astype(np.float64) - 0.5 * c_sq64


def kernel(**inputs):
    x = np.asarray(inputs["input_values"], dtype=np.float32)
    gamma = np.asarray(inputs["ln_gamma"], dtype=np.float32)
    beta = np.asarray(inputs["ln_beta"], dtype=np.float32)
    W = np.asarray(inputs["proj_w"], dtype=np.float32)
    CB = np.asarray(inputs["code_book"], dtype=np.float32)
    E = np.asarray(inputs["enc_w"], dtype=np.float32)
    eb = np.asarray(inputs["enc_b"], dtype=np.float32)

    B, T, Dx = x.shape
    assert Dx == D and W.shape == (Q, D) and CB.shape == (Q, C)
    xf = np.ascontiguousarray(x.reshape(-1, D))
    N = xf.shape[0]
    NCORE = 8
    assert N % (NCORE * TOK_PER_CHUNK) == 0, f"token count {N} not divisible"
    tpc = N // NCORE
    n_chunks = tpc // TOK_PER_CHUNK

    weff = np.ascontiguousarray((W * gamma[None, :]).T)
    eeff = np.ascontiguousarray((E * gamma[None, :]).T)
    wbias = np.ascontiguousarray((W.astype(np.float64) @ beta.astype(np.float64))
                                 .astype(np.float32).reshape(Q // 128, 128).T)
    ebrow = np.ascontiguousarray(
        ((E.astype(np.float64) @ beta.astype(np.float64)) + eb.astype(np.float64))
        .astype(np.float32).reshape(1, Q))
    c_sq64 = (CB.astype(np.float64) ** 2).sum(0)
    ncsqb = np.ascontiguousarray(np.broadcast_to(
        (-0.5 * (c_sq64 - c_sq64.mean())).astype(np.float32)[None, :], (128, C)))

    has_eb = bool(np.any(ebrow))
    nc = _get_module(n_chunks, has_eb=has_eb)
    in_maps = []
    for cidx in range(NCORE):
        in_maps.append({
            "x": np.ascontiguousarray(xf[cidx * tpc:(cidx + 1) * tpc]),
            "weff": weff, "eeff": eeff, "wbias": wbias, "ebrow": ebrow,
            "cb": CB, "ncsqb": ncsqb,
            "ones": np.ones((1, 128), np.float32),
        })

    trace = bool(int(os.environ.get("KERNEL_TRACE", "0")))
    res = bass_utils.run_bass_kernel_spmd(nc, in_maps, core_ids=list(range(NCORE)),
                                          trace=trace)
    kernel.last_results = res

    enc = np.concatenate([np.asarray(r["enc"]) for r in res.results], axis=0)
    aux = np.stack([np.asarray(r["aux"]) for r in res.results], axis=0)  # (8,2,tpc,16)

    v1 = aux[..., 0].astype(np.float64)          # (8, 2, tpc) top-1 per half
    v2 = aux[..., 1].astype(np.float64)          # second-best per half
    idx = np.ascontiguousarray(aux).view(np.uint32)[..., 8].astype(np.int64)

    win = np.argmax(v1, axis=1)                  # (8, tpc) winning half
    ar = np.arange(tpc)
    cr = np.arange(NCORE)[:, None]
    vwin = v1[cr, win, ar]
    vlose = v1[cr, 1 - win, ar]
    v2win = v2[cr, win, ar]
    second = np.maximum(vlose, v2win)
    margin = vwin - second
    labels = (idx[cr, win, ar] + win.astype(np.int64) * CHALF).reshape(-1)

    flagged = (margin < MARGIN).reshape(-1)
    kernel.last_flagged_count = int(flagged.sum())
    if flagged.any():
        fidx = np.nonzero(flagged)[0]
        scores = _host_reference_scores(xf[fidx], gamma, beta, W, CB, c_sq64)
        labels[fidx] = scores.argmax(-1)

    encoder_out = enc.reshape(B, T, Q)
    labels = labels.reshape(B, T).astype(np.int32)
    return encoder_out, labels
